# revision 13
# baseline (speedup 1.0000x reference)
"""Batched int8 GEMM (s8t x s8n -> s32t) on 8 TRN2 NeuronCores.

out[b, m, n] = sum_k a[b, m, k] * b[b, n, k]   (int32 accumulation)
a: [32, 1024, 1024] int8, b: [32, 1024, 1024] int8 -> out: [32, 1024, 1024] int32

Strategy:
  - Pure batch parallelism: 4 batches per core across 8 cores.
  - Both operands have K innermost, but the PE needs K on partitions.
    DMA-transpose works on 2-byte elements only, so we view the int8
    inputs as uint16 (pairs of adjacent K values), DMA-transpose K-blocks
    of 256 K-values into SBUF as [128 part, M] uint16 (each partition
    holds an even/odd K pair, interleaved along the free dim).
  - DVE deinterleaves (stride-2 int8 reads) and converts int8 -> bf16.
    int8 subset of bf16 is exact; products <= 2^14 and sums <= 2^24 are
    exact in fp32 PSUM accumulation, so the GEMM is bit-exact.
  - PE: bf16 matmuls, K=128 per instruction, 8-step accumulation into a
    [128, 512] fp32 PSUM bank.
  - ACT copies PSUM fp32 -> SBUF int32 (exact: values are integers),
    sync-DMA stores to HBM.
"""

import numpy as np

import concourse.bass as bass
import concourse.mybir as mybir
import concourse.tile as tile
from concourse import bacc
from concourse.bass_utils import run_bass_kernel_spmd

B, M, N, K = 32, 1024, 1024, 1024
N_CORES = 8
BPC = B // N_CORES  # batches per core
KB = K // 256  # k-blocks of 256 K-values (128 uint16 partitions)
N_TILE = 512
M_TILE = 128

_nc_cache = None


def build_nc():
    nc = bacc.Bacc("TRN2")

    # int8 inputs viewed as uint16 so the xbar DMA-transpose (2-byte
    # granularity) can be used straight out of HBM.
    a_in = nc.dram_tensor("a", [BPC, M, K // 2], mybir.dt.uint16, kind="ExternalInput")
    b_in = nc.dram_tensor("b", [BPC, N, K // 2], mybir.dt.uint16, kind="ExternalInput")
    out = nc.dram_tensor("out", [BPC, M, N], mybir.dt.int32, kind="ExternalOutput")

    with tile.TileContext(nc) as tc:
        with (
            tc.tile_pool(name="stage", bufs=4) as stage_pool,
            tc.tile_pool(name="conv", bufs=2) as conv_pool,
            tc.tile_pool(name="psum", bufs=8, space="PSUM") as psum_pool,
            tc.tile_pool(name="outbuf", bufs=4) as out_pool,
            tc.tile_pool(name="warm", bufs=1) as warm_pool,
        ):
            # PE warmup: ~4.5us of dummy matmuls with no DMA deps, so the
            # HAM clock gate reaches K=8/8 before the real MM stream starts.
            wsrc = warm_pool.tile([128, N_TILE], mybir.dt.bfloat16, name="wsrc")
            nc.gpsimd.memset(wsrc[:], 0.0)
            wps = psum_pool.tile([128, N_TILE], mybir.dt.float32, name="wps", tag="ps")
            for _ in range(21):
                nc.tensor.matmul(wps[:], wsrc[:, :128], wsrc[:], start=True, stop=True)

            for bi in range(BPC):
                # ---- load k-blocks transposed: [128 part, M] uint16 ----
                a_bf = []  # 8 bf16 tiles [128, M]; k-tile index kb*2+parity
                b_bf = []
                for kb in range(KB):
                    at = stage_pool.tile(
                        [128, M], mybir.dt.uint16, name=f"at_{bi}_{kb}", tag=f"at{kb}"
                    )
                    nc.sync.dma_start_transpose(at[:], a_in[bi, :, kb * 128 : (kb + 1) * 128])
                    bt = stage_pool.tile(
                        [128, N], mybir.dt.uint16, name=f"bt_{bi}_{kb}", tag=f"bt{kb}"
                    )
                    nc.sync.dma_start_transpose(bt[:], b_in[bi, :, kb * 128 : (kb + 1) * 128])

                    # ---- deinterleave + int8 -> bf16 (DVE) ----
                    at8 = at.bitcast(mybir.dt.int8)  # [128, 2M]
                    bt8 = bt.bitcast(mybir.dt.int8)
                    for par in range(2):
                        abf = conv_pool.tile(
                            [128, M],
                            mybir.dt.bfloat16,
                            name=f"abf_{bi}_{kb}_{par}",
                            tag=f"abf{kb}{par}",
                        )
                        nc.vector.tensor_copy(abf[:], at8[:, par::2])
                        a_bf.append(abf)
                        bbf = conv_pool.tile(
                            [128, N],
                            mybir.dt.bfloat16,
                            name=f"bbf_{bi}_{kb}_{par}",
                            tag=f"bbf{kb}{par}",
                        )
                        nc.vector.tensor_copy(bbf[:], bt8[:, par::2])
                        b_bf.append(bbf)

                # ---- GEMM: mt -> kt -> nt, accumulate in PSUM over kt ----
                n_kt = 2 * KB
                for mt in range(M // M_TILE):
                    ps = [
                        psum_pool.tile(
                            [128, N_TILE], mybir.dt.float32, name=f"ps_{bi}_{mt}_{nt}", tag="ps"
                        )
                        for nt in range(N // N_TILE)
                    ]
                    for kt in range(n_kt):
                        lhsT = a_bf[kt][:, mt * M_TILE : (mt + 1) * M_TILE]
                        for nt in range(N // N_TILE):
                            nc.tensor.matmul(
                                ps[nt][:],
                                lhsT,
                                b_bf[kt][:, nt * N_TILE : (nt + 1) * N_TILE],
                                start=(kt == 0),
                                stop=(kt == n_kt - 1),
                            )
                    # Engine separation so no FIFO head-of-line blocking can
                    # couple the pipelines: DVE does only deints (feeds PE),
                    # ACT does only PSUM-freeing fp32->int32 copies, GPSIMD
                    # (SWDGE) does stores (off nc.sync, so no xbar-mode drain
                    # against the transposes), SYNC does only transposes.
                    ot = out_pool.tile([128, N], mybir.dt.int32, name=f"ot_{bi}_{mt}", tag="ot")
                    for nt in range(N // N_TILE):
                        nc.scalar.copy(
                            ot[:, nt * N_TILE : (nt + 1) * N_TILE], ps[nt][:]
                        )
                    nc.gpsimd.dma_start(
                        out[bi, mt * M_TILE : (mt + 1) * M_TILE, :], ot[:]
                    )
    nc.compile()
    return nc


def _get_nc():
    global _nc_cache
    if _nc_cache is None:
        _nc_cache = build_nc()
    return _nc_cache


def run(a: np.ndarray, b: np.ndarray, trace: bool = False):
    """Run on 8 cores. a/b: [32, 1024, 1024] int8. Returns (out, BassKernelResults)."""
    a = np.ascontiguousarray(a)
    b = np.ascontiguousarray(b)
    a16 = a.view(np.uint16).reshape(B, M, K // 2)
    b16 = b.view(np.uint16).reshape(B, N, K // 2)
    in_maps = [
        {
            "a": a16[c * BPC : (c + 1) * BPC],
            "b": b16[c * BPC : (c + 1) * BPC],
        }
        for c in range(N_CORES)
    ]
    res = run_bass_kernel_spmd(_get_nc(), in_maps, list(range(N_CORES)), trace=trace)
    out = np.concatenate([res.results[c]["out"] for c in range(N_CORES)], axis=0)
    return out, res


def kernel(a: np.ndarray, b: np.ndarray) -> np.ndarray:
    out, _ = run(np.asarray(a), np.asarray(b))
    return out


# revision 14
# speedup vs baseline: 1.0222x; 1.0222x over previous
"""Batched int8 GEMM (s8t x s8n -> s32t) on 8 TRN2 NeuronCores.

out[b, m, n] = sum_k a[b, m, k] * b[b, n, k]   (int32 accumulation)
a: [32, 1024, 1024] int8, b: [32, 1024, 1024] int8 -> out: [32, 1024, 1024] int32

Strategy:
  - Pure batch parallelism: 4 batches per core across 8 cores.
  - Both operands have K innermost, but the PE needs K on partitions.
    DMA-transpose works on 2-byte elements only, so we view the int8
    inputs as uint16 (pairs of adjacent K values), DMA-transpose K-blocks
    of 256 K-values into SBUF as [128 part, M] uint16 (each partition
    holds an even/odd K pair, interleaved along the free dim).
  - DVE deinterleaves (stride-2 int8 reads) and converts int8 -> bf16.
    int8 subset of bf16 is exact; products <= 2^14 and sums <= 2^24 are
    exact in fp32 PSUM accumulation, so the GEMM is bit-exact.
  - PE: bf16 matmuls, K=128 per instruction, 8-step accumulation into a
    [128, 512] fp32 PSUM bank.
  - ACT copies PSUM fp32 -> SBUF int32 (exact: values are integers),
    sync-DMA stores to HBM.
"""

import numpy as np

import concourse.bass as bass
import concourse.mybir as mybir
import concourse.tile as tile
from concourse import bacc
from concourse.bass_utils import run_bass_kernel_spmd

B, M, N, K = 32, 1024, 1024, 1024
N_CORES = 8
BPC = B // N_CORES  # batches per core
KB = K // 256  # k-blocks of 256 K-values (128 uint16 partitions)
N_TILE = 512
M_TILE = 128

_nc_cache = None


def build_nc():
    nc = bacc.Bacc("TRN2")

    # int8 inputs viewed as uint16 so the xbar DMA-transpose (2-byte
    # granularity) can be used straight out of HBM.
    a_in = nc.dram_tensor("a", [BPC, M, K // 2], mybir.dt.uint16, kind="ExternalInput")
    b_in = nc.dram_tensor("b", [BPC, N, K // 2], mybir.dt.uint16, kind="ExternalInput")
    out = nc.dram_tensor("out", [BPC, M, N], mybir.dt.int32, kind="ExternalOutput")

    with tile.TileContext(nc) as tc:
        with (
            tc.tile_pool(name="stage", bufs=4) as stage_pool,
            tc.tile_pool(name="conv", bufs=2) as conv_pool,
            tc.tile_pool(name="psum", bufs=8, space="PSUM") as psum_pool,
            tc.tile_pool(name="outbuf", bufs=4) as out_pool,
            tc.tile_pool(name="warm", bufs=1) as warm_pool,
        ):
            # PE warmup: ~4.5us of dummy matmuls with no DMA deps, so the
            # HAM clock gate reaches K=8/8 before the real MM stream starts.
            wsrc = warm_pool.tile([128, N_TILE], mybir.dt.bfloat16, name="wsrc")
            nc.gpsimd.memset(wsrc[:], 0.0)
            wps = psum_pool.tile([128, N_TILE], mybir.dt.float32, name="wps", tag="ps")
            for _ in range(21):
                nc.tensor.matmul(wps[:], wsrc[:, :128], wsrc[:], start=True, stop=True)

            for bi in range(BPC):
                # ---- load k-blocks transposed: [128 part, M] uint16 ----
                a_bf = []  # 8 bf16 tiles [128, M]; k-tile index kb*2+parity
                b_bf = []
                for kb in range(KB):
                    at = stage_pool.tile(
                        [128, M], mybir.dt.uint16, name=f"at_{bi}_{kb}", tag=f"at{kb}"
                    )
                    nc.sync.dma_start_transpose(at[:], a_in[bi, :, kb * 128 : (kb + 1) * 128])
                    bt = stage_pool.tile(
                        [128, N], mybir.dt.uint16, name=f"bt_{bi}_{kb}", tag=f"bt{kb}"
                    )
                    nc.sync.dma_start_transpose(bt[:], b_in[bi, :, kb * 128 : (kb + 1) * 128])

                    # ---- deinterleave + int8 -> bf16 (DVE) ----
                    at8 = at.bitcast(mybir.dt.int8)  # [128, 2M]
                    bt8 = bt.bitcast(mybir.dt.int8)
                    for par in range(2):
                        abf = conv_pool.tile(
                            [128, M],
                            mybir.dt.bfloat16,
                            name=f"abf_{bi}_{kb}_{par}",
                            tag=f"abf{kb}{par}",
                        )
                        nc.vector.tensor_copy(abf[:], at8[:, par::2])
                        a_bf.append(abf)
                        bbf = conv_pool.tile(
                            [128, N],
                            mybir.dt.bfloat16,
                            name=f"bbf_{bi}_{kb}_{par}",
                            tag=f"bbf{kb}{par}",
                        )
                        nc.vector.tensor_copy(bbf[:], bt8[:, par::2])
                        b_bf.append(bbf)

                # ---- GEMM: mt -> kt -> nt, accumulate in PSUM over kt ----
                n_kt = 2 * KB
                for mt in range(M // M_TILE):
                    ps = [
                        psum_pool.tile(
                            [128, N_TILE], mybir.dt.float32, name=f"ps_{bi}_{mt}_{nt}", tag="ps"
                        )
                        for nt in range(N // N_TILE)
                    ]
                    for kt in range(n_kt):
                        lhsT = a_bf[kt][:, mt * M_TILE : (mt + 1) * M_TILE]
                        for nt in range(N // N_TILE):
                            nc.tensor.matmul(
                                ps[nt][:],
                                lhsT,
                                b_bf[kt][:, nt * N_TILE : (nt + 1) * N_TILE],
                                start=(kt == 0),
                                stop=(kt == n_kt - 1),
                            )
                    # Engine separation so no FIFO head-of-line blocking can
                    # couple the pipelines: DVE does only deints (feeds PE),
                    # ACT does only PSUM-freeing fp32->int32 copies, GPSIMD
                    # (SWDGE) does stores (off nc.sync, so no xbar-mode drain
                    # against the transposes), SYNC does only transposes.
                    ot = out_pool.tile([128, N], mybir.dt.int32, name=f"ot_{bi}_{mt}", tag="ot")
                    for nt in range(N // N_TILE):
                        nc.scalar.copy(
                            ot[:, nt * N_TILE : (nt + 1) * N_TILE], ps[nt][:]
                        )
                    nc.scalar.dma_start(
                        out[bi, mt * M_TILE : (mt + 1) * M_TILE, :], ot[:]
                    )
    nc.compile()
    return nc


def _get_nc():
    global _nc_cache
    if _nc_cache is None:
        _nc_cache = build_nc()
    return _nc_cache


def run(a: np.ndarray, b: np.ndarray, trace: bool = False):
    """Run on 8 cores. a/b: [32, 1024, 1024] int8. Returns (out, BassKernelResults)."""
    a = np.ascontiguousarray(a)
    b = np.ascontiguousarray(b)
    a16 = a.view(np.uint16).reshape(B, M, K // 2)
    b16 = b.view(np.uint16).reshape(B, N, K // 2)
    in_maps = [
        {
            "a": a16[c * BPC : (c + 1) * BPC],
            "b": b16[c * BPC : (c + 1) * BPC],
        }
        for c in range(N_CORES)
    ]
    res = run_bass_kernel_spmd(_get_nc(), in_maps, list(range(N_CORES)), trace=trace)
    out = np.concatenate([res.results[c]["out"] for c in range(N_CORES)], axis=0)
    return out, res


def kernel(a: np.ndarray, b: np.ndarray) -> np.ndarray:
    out, _ = run(np.asarray(a), np.asarray(b))
    return out


# revision 15
# speedup vs baseline: 1.1043x; 1.0803x over previous
"""Batched int8 GEMM (s8t x s8n -> s32t) on 8 TRN2 NeuronCores.

out[b, m, n] = sum_k a[b, m, k] * b[b, n, k]   (int32 accumulation)
a: [32, 1024, 1024] int8, b: [32, 1024, 1024] int8 -> out: [32, 1024, 1024] int32

Strategy:
  - Pure batch parallelism: 4 batches per core across 8 cores.
  - Both operands have K innermost, but the PE needs K on partitions.
    DMA-transpose works on 2-byte elements only, so we view the int8
    inputs as uint16 (pairs of adjacent K values) and DMA-transpose
    K-blocks of 256 K-values for a *pair of batches* at once
    ([2048, 128] uint16 -> [128, 2048]), each partition holding an
    even/odd K pair interleaved along the free dim.
  - DVE deinterleaves (stride-2 int8 reads) and converts int8 -> bf16.
    int8 is exactly representable in bf16; products <= 2^14 and sums
    <= 2^24 are exact in fp32 PSUM accumulation, so the GEMM is
    bit-exact.
  - PE: bf16 matmuls, K=128 per instruction, 8-step accumulation into
    [128, 512] fp32 PSUM banks. ~21 dummy matmuls up front warm the HAM
    clock gate before the real stream arrives.
  - ACT copies PSUM fp32 -> SBUF int32 (exact: values are integers) and
    issues the output stores (HWDGE); SYNC issues only transposes. The
    strict engine separation avoids FIFO head-of-line blocking between
    the deint stream, the PSUM-freeing stream, and the DMA streams.
"""

import numpy as np

import concourse.bass as bass
import concourse.mybir as mybir
import concourse.tile as tile
from concourse import bacc
from concourse.bass_utils import run_bass_kernel_spmd

B, M, N, K = 32, 1024, 1024, 1024
N_CORES = 8
BPC = B // N_CORES  # batches per core
KB = K // 256  # k-blocks of 256 K-values (128 uint16 partitions)
N_TILE = 512
M_TILE = 128

_nc_cache = None


def build_nc():
    nc = bacc.Bacc("TRN2")

    # int8 inputs viewed as uint16 so the xbar DMA-transpose (2-byte
    # granularity) can be used straight out of HBM.
    a_in = nc.dram_tensor("a", [BPC, M, K // 2], mybir.dt.uint16, kind="ExternalInput")
    b_in = nc.dram_tensor("b", [BPC, N, K // 2], mybir.dt.uint16, kind="ExternalInput")
    out = nc.dram_tensor("out", [BPC, M, N], mybir.dt.int32, kind="ExternalOutput")

    with tile.TileContext(nc) as tc:
        with (
            tc.tile_pool(name="stage", bufs=2) as stage_pool,
            tc.tile_pool(name="conv", bufs=2) as conv_pool,
            tc.tile_pool(name="psum", bufs=8, space="PSUM") as psum_pool,
            tc.tile_pool(name="outbuf", bufs=4) as out_pool,
            tc.tile_pool(name="warm", bufs=1) as warm_pool,
        ):
            # PE warmup: ~4.5us of dummy matmuls with no DMA deps, so the
            # HAM clock gate reaches K=8/8 before the real MM stream starts.
            wsrc = warm_pool.tile([128, N_TILE], mybir.dt.bfloat16, name="wsrc")
            nc.gpsimd.memset(wsrc[:], 0.0)
            wps = psum_pool.tile([128, N_TILE], mybir.dt.float32, name="wps", tag="ps")
            for _ in range(21):
                nc.tensor.matmul(wps[:], wsrc[:, :128], wsrc[:], start=True, stop=True)

            for bp in range(BPC // 2):  # batch pairs
                # ---- one transpose per (operand, k-block) covers 2 batches:
                # [2*M rows, 128 uint16] -> [128 part, 2*M] ----
                a_st = []
                b_st = []
                for kb in range(KB):
                    at = stage_pool.tile(
                        [128, 2 * M], mybir.dt.uint16, name=f"at_{bp}_{kb}", tag=f"at{kb}"
                    )
                    nc.sync.dma_start_transpose(
                        at[:],
                        a_in[2 * bp : 2 * bp + 2, :, kb * 128 : (kb + 1) * 128].rearrange(
                            "b m k -> (b m) k"
                        ),
                    )
                    a_st.append(at.bitcast(mybir.dt.int8))  # [128, 4M]
                    bt = stage_pool.tile(
                        [128, 2 * N], mybir.dt.uint16, name=f"bt_{bp}_{kb}", tag=f"bt{kb}"
                    )
                    nc.sync.dma_start_transpose(
                        bt[:],
                        b_in[2 * bp : 2 * bp + 2, :, kb * 128 : (kb + 1) * 128].rearrange(
                            "b m k -> (b m) k"
                        ),
                    )
                    b_st.append(bt.bitcast(mybir.dt.int8))

                for half in range(2):  # batch within the pair
                    bi = 2 * bp + half
                    # ---- deinterleave + int8 -> bf16 (DVE) ----
                    a_bf = []  # 8 bf16 tiles [128, M]; k-tile = kb*2+parity
                    b_bf = []
                    for kb in range(KB):
                        for par in range(2):
                            abf = conv_pool.tile(
                                [128, M],
                                mybir.dt.bfloat16,
                                name=f"abf_{bi}_{kb}_{par}",
                                tag=f"abf{kb}{par}",
                            )
                            nc.vector.tensor_copy(
                                abf[:], a_st[kb][:, 2 * M * half + par : 2 * M * (half + 1) : 2]
                            )
                            a_bf.append(abf)
                            bbf = conv_pool.tile(
                                [128, N],
                                mybir.dt.bfloat16,
                                name=f"bbf_{bi}_{kb}_{par}",
                                tag=f"bbf{kb}{par}",
                            )
                            nc.vector.tensor_copy(
                                bbf[:], b_st[kb][:, 2 * N * half + par : 2 * N * (half + 1) : 2]
                            )
                            b_bf.append(bbf)

                    # ---- GEMM: mt -> kt -> nt, accumulate in PSUM over kt;
                    # stores merged over mt pairs ----
                    n_kt = 2 * KB
                    for mt2 in range(M // M_TILE // 2):
                        ot = out_pool.tile(
                            [128, 2, N], mybir.dt.int32, name=f"ot_{bi}_{mt2}", tag="ot"
                        )
                        for sub in range(2):
                            mt = 2 * mt2 + sub
                            ps = [
                                psum_pool.tile(
                                    [128, N_TILE],
                                    mybir.dt.float32,
                                    name=f"ps_{bi}_{mt}_{nt}",
                                    tag="ps",
                                )
                                for nt in range(N // N_TILE)
                            ]
                            for kt in range(n_kt):
                                lhsT = a_bf[kt][:, mt * M_TILE : (mt + 1) * M_TILE]
                                for nt in range(N // N_TILE):
                                    nc.tensor.matmul(
                                        ps[nt][:],
                                        lhsT,
                                        b_bf[kt][:, nt * N_TILE : (nt + 1) * N_TILE],
                                        start=(kt == 0),
                                        stop=(kt == n_kt - 1),
                                    )
                            # fp32 -> int32 PSUM-freeing copies on ACT.
                            for nt in range(N // N_TILE):
                                nc.scalar.copy(
                                    ot[:, sub, nt * N_TILE : (nt + 1) * N_TILE], ps[nt][:]
                                )
                        # One 1MiB store for both mt blocks: HBM rows
                        # (sub*128 + p) paired with SBUF [p, sub, :].
                        nc.scalar.dma_start(
                            out[bi, mt2 * 256 : (mt2 + 1) * 256, :].rearrange(
                                "(s p) n -> p s n", s=2
                            ),
                            ot[:],
                        )
    nc.compile()
    return nc


def _get_nc():
    global _nc_cache
    if _nc_cache is None:
        _nc_cache = build_nc()
    return _nc_cache


def run(a: np.ndarray, b: np.ndarray, trace: bool = False):
    """Run on 8 cores. a/b: [32, 1024, 1024] int8. Returns (out, BassKernelResults)."""
    a = np.ascontiguousarray(a)
    b = np.ascontiguousarray(b)
    a16 = a.view(np.uint16).reshape(B, M, K // 2)
    b16 = b.view(np.uint16).reshape(B, N, K // 2)
    in_maps = [
        {
            "a": a16[c * BPC : (c + 1) * BPC],
            "b": b16[c * BPC : (c + 1) * BPC],
        }
        for c in range(N_CORES)
    ]
    res = run_bass_kernel_spmd(_get_nc(), in_maps, list(range(N_CORES)), trace=trace)
    out = np.concatenate([res.results[c]["out"] for c in range(N_CORES)], axis=0)
    return out, res


def kernel(a: np.ndarray, b: np.ndarray) -> np.ndarray:
    out, _ = run(np.asarray(a), np.asarray(b))
    return out
